# revision 4
# baseline (speedup 1.0000x reference)
"""Trainium2 Bass kernel for 16-head MHA (B=2, N=2048, D=1024, H=16).

Sharding: 8 cores = batch(2) x head-group(4). Each core computes 4 heads of
one batch element end-to-end (QKV projections, attention, and its partial
contribution to the output projection). The output projection is a sum over
head features, so each core returns a (N, D) partial product; the host sums
the 4 partials per batch and adds the output bias during unshard.

Per-core dataflow (all matmuls bf16 inputs, f32 PSUM accumulation):
  qT/kT = W @ x.T          (head-feature-major layout, 2 tiles of (128, N))
  v     = x @ Wv.T + bv    stored per key-tile as [v_h | ones] blocks
  scoresT[kt] = k @ q.T    (keys on partitions -> softmax denom comes from PE)
  expT = exp(SCALE*scoresT)  on ScalarE, reading PSUM directly
  [yT; denom] += [v|1].T @ expT  accumulated over key tiles
  yT_norm = yT * 1/denom   (denominator broadcast across partitions by PE)
  partial = yT_norm.T @ WoT
Heads are processed in pairs occupying partitions 0-63 / 64-127 so the two
scoresT matmuls (contraction K=64) row-pack onto disjoint PE row groups.
"""

import os
import sys
from contextlib import ExitStack

import numpy as np

if "/opt/trn_rl_repo" not in sys.path:
    sys.path.insert(0, "/opt/trn_rl_repo")

import ml_dtypes

P = 128
B = 2
NTOK = 2048  # sequence length
D = 1024  # model dim
H_PER_CORE = 4  # heads per core
HD = 64  # head dim
DG = H_PER_CORE * HD  # head-group feature width per core (256)
QB = 512  # query block (matmul free dim)
N_QB = NTOK // QB  # 4
N_KT = NTOK // P  # 16 key tiles
N_DT = D // P  # 8 contraction tiles for projections
SCALE = HD ** -0.5

_BF16 = ml_dtypes.bfloat16


def _emit(tc, t):
    import concourse.bass as bass
    from concourse import mybir

    F32 = mybir.dt.float32
    BF16 = mybir.dt.bfloat16
    Exp = mybir.ActivationFunctionType.Exp
    nc = tc.nc

    with ExitStack() as ctx:
        consts = ctx.enter_context(tc.tile_pool(name="consts", bufs=1))

        # ---- resident SBUF tensors ----
        xT_t = [consts.tile([P, NTOK], BF16, tag=f"xT{i}", name=f"xT{i}") for i in range(N_DT)]
        wqT_t = [consts.tile([P, DG], BF16, tag=f"wqT{i}", name=f"wqT{i}") for i in range(N_DT)]
        wkT_t = [consts.tile([P, DG], BF16, tag=f"wkT{i}", name=f"wkT{i}") for i in range(N_DT)]
        wvT_t = [consts.tile([P, DG], BF16, tag=f"wvT{i}", name=f"wvT{i}") for i in range(N_DT)]
        woT_t = [consts.tile([P, D], BF16, tag=f"woT{i}", name=f"woT{i}") for i in range(DG // P)]
        bq_t = [consts.tile([P, 1], F32, tag=f"bq{i}", name=f"bq{i}") for i in range(DG // P)]
        bk_t = [consts.tile([P, 1], F32, tag=f"bk{i}", name=f"bk{i}") for i in range(DG // P)]
        bvb_t = consts.tile([P, DG], F32, tag="bvb", name="bvb")
        qT_t = [consts.tile([P, NTOK], BF16, tag=f"qT{i}", name=f"qT{i}") for i in range(DG // P)]
        kT_t = [consts.tile([P, NTOK], BF16, tag=f"kT{i}", name=f"kT{i}") for i in range(DG // P)]
        # v per key tile: 4 head blocks of [v_h (64 cols) | ones (64 cols)]
        v_t = [
            consts.tile([P, H_PER_CORE * 2 * HD], BF16, tag=f"v{i}", name=f"v{i}")
            for i in range(N_KT)
        ]
        yT_t = [consts.tile([P, NTOK], BF16, tag=f"yT{i}", name=f"yT{i}") for i in range(DG // P)]

        # ---- input DMAs ----
        for i in range(N_DT):
            nc.sync.dma_start(xT_t[i][:], t["xT"][i * P : (i + 1) * P, :])
            nc.sync.dma_start(wqT_t[i][:], t["wqT"][i * P : (i + 1) * P, :])
            nc.sync.dma_start(wkT_t[i][:], t["wkT"][i * P : (i + 1) * P, :])
            nc.sync.dma_start(wvT_t[i][:], t["wvT"][i * P : (i + 1) * P, :])
        for i in range(DG // P):
            nc.sync.dma_start(woT_t[i][:], t["woT"][i * P : (i + 1) * P, :])
            nc.sync.dma_start(bq_t[i][:], t["bq2"][i])
            nc.sync.dma_start(bk_t[i][:], t["bk2"][i])
        nc.sync.dma_start(bvb_t[:], t["bvb"][:])

        # ---- phase 1: projections ----
        with tc.tile_pool(name="proj_psum", bufs=4, space="PSUM") as proj_psum:
            # qT / kT: out (DG, NTOK) = W @ x.T, feature-major
            for wt, bt, dst in ((wqT_t, bq_t, qT_t), (wkT_t, bk_t, kT_t)):
                for pt in range(DG // P):
                    for qb in range(N_QB):
                        pp = proj_psum.tile([P, QB], F32, tag="pp_qk", name="pp_qk")
                        for dt in range(N_DT):
                            nc.tensor.matmul(
                                pp[:],
                                lhsT=wt[dt][:, pt * P : (pt + 1) * P],
                                rhs=xT_t[dt][:, qb * QB : (qb + 1) * QB],
                                start=(dt == 0),
                                stop=(dt == N_DT - 1),
                            )
                        nc.vector.tensor_scalar_add(
                            dst[pt][:, qb * QB : (qb + 1) * QB], pp[:], bt[pt][:]
                        )
            # v: out per key tile (128 tokens, DG feats) = x @ Wv.T, token-major
            for kt in range(N_KT):
                pp = proj_psum.tile([P, DG], F32, tag="pp_v", name="pp_v")
                for dt in range(N_DT):
                    nc.tensor.matmul(
                        pp[:],
                        lhsT=xT_t[dt][:, kt * P : (kt + 1) * P],
                        rhs=wvT_t[dt][:],
                        start=(dt == 0),
                        stop=(dt == N_DT - 1),
                    )
                vk = v_t[kt].rearrange("p (h w) -> p h w", h=H_PER_CORE)
                nc.vector.tensor_add(
                    vk[:, :, 0:HD],
                    pp[:].rearrange("p (h w) -> p h w", h=H_PER_CORE),
                    bvb_t[:].rearrange("p (h w) -> p h w", h=H_PER_CORE),
                )
                nc.vector.memset(vk[:, :, HD : 2 * HD], 1.0)

        # ---- phase 2: attention, head pairs row-packed on the PE ----
        with (
            tc.tile_pool(name="sc_psum", bufs=2, space="PSUM") as sc_psum,
            tc.tile_pool(name="pv_psum", bufs=2, space="PSUM") as pv_psum,
            tc.tile_pool(name="expT", bufs=4) as expT_pool,
            tc.tile_pool(name="rcp", bufs=3) as rcp_pool,
        ):
            for pair in range(H_PER_CORE // 2):
                ha, hb = 2 * pair, 2 * pair + 1
                kt_pair = kT_t[pair]
                qt_pair = qT_t[pair]
                for qb in range(N_QB):
                    qsl = slice(qb * QB, (qb + 1) * QB)
                    pv_a = pv_psum.tile([P, QB], F32, tag="pv_a", name="pv_a")
                    pv_b = pv_psum.tile([P, QB], F32, tag="pv_b", name="pv_b")
                    for kt in range(N_KT):
                        ksl = slice(kt * P, (kt + 1) * P)
                        sc = sc_psum.tile([P, 2 * QB], F32, tag="sc", name="sc")
                        # scoresT = k @ q.T, keys on partitions; the two heads
                        # land on PE row groups 0-63 / 64-127 concurrently
                        nc.tensor.matmul(
                            sc[:, 0:QB],
                            lhsT=kt_pair[0:HD, ksl],
                            rhs=qt_pair[0:HD, qsl],
                            start=True,
                            stop=True,
                        )
                        nc.tensor.matmul(
                            sc[:, QB : 2 * QB],
                            lhsT=kt_pair[HD:P, ksl],
                            rhs=qt_pair[HD:P, qsl],
                            start=True,
                            stop=True,
                        )
                        ex = expT_pool.tile([P, 2 * QB], BF16, tag="ex", name="ex")
                        nc.scalar.activation(ex[:], sc[:], Exp, scale=SCALE)
                        # [yT; denom-broadcast] accumulation over key tiles
                        nc.tensor.matmul(
                            pv_a[:],
                            lhsT=v_t[kt][:, ha * 2 * HD : (ha + 1) * 2 * HD],
                            rhs=ex[:, 0:QB],
                            start=(kt == 0),
                            stop=(kt == N_KT - 1),
                        )
                        nc.tensor.matmul(
                            pv_b[:],
                            lhsT=v_t[kt][:, hb * 2 * HD : (hb + 1) * 2 * HD],
                            rhs=ex[:, QB : 2 * QB],
                            start=(kt == 0),
                            stop=(kt == N_KT - 1),
                        )
                    # normalize: yT rows 0-63, denominator rows 64-127
                    for h, pv in ((ha, pv_a), (hb, pv_b)):
                        rc = rcp_pool.tile([P, QB], F32, tag="rc", name="rc")
                        nc.vector.reciprocal(rc[HD:P, :], pv[HD:P, :])
                        po = (h % 2) * HD
                        nc.vector.tensor_mul(
                            yT_t[h // 2][po : po + HD, qsl],
                            pv[0:HD, :],
                            rc[HD:P, :],
                        )

        # ---- phase 3: output projection partial = yT.T @ WoT ----
        with (
            tc.tile_pool(name="op_psum", bufs=4, space="PSUM") as op_psum,
            tc.tile_pool(name="ob", bufs=3) as ob_pool,
        ):
            for mt in range(N_KT):
                msl = slice(mt * P, (mt + 1) * P)
                ob = ob_pool.tile([P, D], F32, tag="ob", name="ob")
                for nb in range(D // QB):
                    op = op_psum.tile([P, QB], F32, tag="op", name="op")
                    for ktile in range(DG // P):
                        nc.tensor.matmul(
                            op[:],
                            lhsT=yT_t[ktile][:, msl],
                            rhs=woT_t[ktile][:, nb * QB : (nb + 1) * QB],
                            start=(ktile == 0),
                            stop=(ktile == DG // P - 1),
                        )
                    nc.any.tensor_copy(ob[:, nb * QB : (nb + 1) * QB], op[:])
                nc.sync.dma_start(t["partial"][msl, :], ob[:])


def _build():
    import concourse.bacc as bacc
    import concourse.tile as tile
    from concourse import mybir

    F32 = mybir.dt.float32
    BF16 = mybir.dt.bfloat16

    nc = bacc.Bacc(
        "TRN2", target_bir_lowering=False, debug=False, num_devices=8
    )
    t = {
        "xT": nc.dram_tensor("xT", (D, NTOK), BF16, kind="ExternalInput").ap(),
        "wqT": nc.dram_tensor("wqT", (D, DG), BF16, kind="ExternalInput").ap(),
        "wkT": nc.dram_tensor("wkT", (D, DG), BF16, kind="ExternalInput").ap(),
        "wvT": nc.dram_tensor("wvT", (D, DG), BF16, kind="ExternalInput").ap(),
        "woT": nc.dram_tensor("woT", (DG, D), BF16, kind="ExternalInput").ap(),
        "bq2": nc.dram_tensor(
            "bq2", (DG // P, P, 1), F32, kind="ExternalInput"
        ).ap(),
        "bk2": nc.dram_tensor(
            "bk2", (DG // P, P, 1), F32, kind="ExternalInput"
        ).ap(),
        "bvb": nc.dram_tensor("bvb", (P, DG), F32, kind="ExternalInput").ap(),
        "partial": nc.dram_tensor(
            "partial", (NTOK, D), F32, kind="ExternalOutput"
        ).ap(),
    }
    with tile.TileContext(nc) as tc:
        _emit(tc, t)
    nc.compile()
    return nc


_CACHE = {}


def _get_nc():
    if "nc" not in _CACHE:
        _CACHE["nc"] = _build()
    return _CACHE["nc"]


def make_in_maps(x, Wq, bq, Wk, bk, Wv, bv, Wo):
    """Per-core host-side sharding: core c -> batch c//4, head group c%4."""
    in_maps = []
    for c in range(8):
        b, g = divmod(c, 4)
        sl = slice(DG * g, DG * (g + 1))
        in_maps.append(
            {
                "xT": np.ascontiguousarray(x[b].T).astype(_BF16),
                "wqT": np.ascontiguousarray(Wq[sl].T).astype(_BF16),
                "wkT": np.ascontiguousarray(Wk[sl].T).astype(_BF16),
                "wvT": np.ascontiguousarray(Wv[sl].T).astype(_BF16),
                "woT": np.ascontiguousarray(Wo[:, sl].T).astype(_BF16),
                "bq2": np.ascontiguousarray(
                    bq[sl].reshape(DG // P, P, 1)
                ).astype(np.float32),
                "bk2": np.ascontiguousarray(
                    bk[sl].reshape(DG // P, P, 1)
                ).astype(np.float32),
                "bvb": np.ascontiguousarray(
                    np.broadcast_to(bv[sl][None, :], (P, DG))
                ).astype(np.float32),
            }
        )
    return in_maps


def kernel(x, Wq, bq, Wk, bk, Wv, bv, Wo, bo, _spmd_kwargs=None):
    from concourse.bass_utils import run_bass_kernel_spmd

    x, Wq, bq, Wk, bk, Wv, bv, Wo, bo = (
        np.asarray(a, np.float32) for a in (x, Wq, bq, Wk, bk, Wv, bv, Wo, bo)
    )
    nc = _get_nc()
    in_maps = make_in_maps(x, Wq, bq, Wk, bk, Wv, bv, Wo)
    res = run_bass_kernel_spmd(
        nc, in_maps, list(range(8)), **(_spmd_kwargs or {})
    )
    _CACHE["last_results"] = res
    out = np.empty((B, NTOK, D), np.float32)
    for b in range(B):
        acc = res.results[4 * b]["partial"].astype(np.float32).copy()
        for g in range(1, 4):
            acc += res.results[4 * b + g]["partial"]
        out[b] = acc + bo[None, :]
    return out
